# revision 1
# baseline (speedup 1.0000x reference)
"""Trainium2 Bass kernel: 4-layer alternating-direction LSTM encoder with
per-layer FFN.  Data-parallel over batch across 8 NeuronCores.

Layout notes (everything "transposed", feature dim on partitions):
 - tokens are interleaved (t*2 + b) on the free axis, b in {0,1} local batch.
 - gate order in packed tiles is [i | f | o | g], each 4 chunks of 128 rows,
   so sigmoid covers cols 0:24 and tanh cols 24:32 of the per-step [128,32]
   gate tile (col = p*2 + b for M-ORDER block p).
 - sequence flips (odd layers) are handled purely by negated dynamic offsets
   in the scan loop; all GEMM phases are direction-agnostic.
"""

import os
import sys

sys.path.insert(0, "/opt/trn_rl_repo")

import numpy as np
import ml_dtypes

import concourse.bass as bass
import concourse.bacc as bacc
import concourse.tile as tile
from concourse import mybir
from concourse.bass import ds

FP32 = mybir.dt.float32
BF16 = mybir.dt.bfloat16

L, H, F = 4, 512, 2048
B, T = 16, 512
NCORES = 8
BL = B // NCORES  # local batch per core
P = 128
KC = H // P  # 4 contraction chunks of H
MB = 4 * H // P  # 16 gate blocks
FB = F // P  # 16 filter blocks
HB = H // P  # 4 hidden blocks
# M-ORDER: natural [i, f, g, o]; o-blocks are emitted last in the scan so
# the i/f/g gate math can start while the o matmuls still run.
ORIG_BLOCK = list(range(16))
SC_I, SC_F, SC_G, SC_O = slice(0, 8), slice(8, 16), slice(16, 24), slice(24, 32)


def _build_nc(T_steps: int, n_layers: int, unroll: int = 8) -> bass.Bass:
    """Build the per-core Bass program (identical on all cores)."""
    NTOK = BL * T_steps
    # token free-dim slices for the big GEMMs (<=512 per matmul)
    NSL = []
    off = 0
    while off < NTOK:
        sz = min(512, NTOK - off)
        NSL.append((off, sz))
        off += sz

    nc = bacc.Bacc()

    xt_d = nc.dram_tensor("xt", [HB, P, NTOK], FP32, kind="ExternalInput")
    whb_d = nc.dram_tensor("whb", [n_layers, P, KC * MB * P], BF16, kind="ExternalInput")
    wxb_d = nc.dram_tensor("wxb", [n_layers, P, KC * MB * P], FP32, kind="ExternalInput")
    w1b_d = nc.dram_tensor("w1b", [n_layers, P, KC * FB * P], FP32, kind="ExternalInput")
    w2b_d = nc.dram_tensor("w2b", [n_layers, P, FB * HB * P], FP32, kind="ExternalInput")
    bb_d = nc.dram_tensor("bb", [n_layers, P, MB], FP32, kind="ExternalInput")
    b1b_d = nc.dram_tensor("b1b", [n_layers, P, FB], FP32, kind="ExternalInput")
    b2b_d = nc.dram_tensor("b2b", [n_layers, P, HB], FP32, kind="ExternalInput")
    out_d = nc.dram_tensor("out", [HB, P, NTOK], FP32, kind="ExternalOutput")

    with tile.TileContext(nc) as tc:
        with (
            tc.tile_pool(name="state", bufs=1) as state,
            tc.tile_pool(name="tmp", bufs=3) as tmp,
            tc.tile_pool(name="psumG", bufs=2, space="PSUM") as pp,
            tc.tile_pool(name="psumS", bufs=2, space="PSUM") as ps,
        ):
            slotA = state.tile([P, HB * NTOK], FP32, tag="slotA")
            slotB = state.tile([P, HB * NTOK], FP32, tag="slotB")
            arena = state.tile([P, T_steps * 2 * MB], FP32, tag="arena")  # xg | z
            h_all = state.tile([P, HB * NTOK], FP32, tag="h_all")
            wbuf = state.tile([P, KC * MB * P], FP32, tag="wbuf")
            whb_sb = state.tile([P, KC * MB * P], BF16, tag="whb_sb")
            bias_sb = state.tile([P, MB], FP32, tag="bias_sb")
            b1_sb = state.tile([P, FB], FP32, tag="b1_sb")
            b2_sb = state.tile([P, HB], FP32, tag="b2_sb")
            hT16 = state.tile([P, 2 * HB], BF16, tag="hT16")
            # cell state lives in PSUM: ACT reads PSUM ~150ns cheaper than SBUF
            cps = ps.tile([P, 2 * HB], FP32, tag="cps", bufs=1)

            def warm(buf):
                # Dummy matmul reading only `buf`: makes the PE observe the
                # buffer's DMA semaphore so real matmuls don't accumulate
                # more sync-waits than the LDWEIGHTS ISA slot budget.
                wp = ps.tile([P, 2], FP32, tag="warmps", bufs=1)
                w = min(buf.shape[1], P)
                nc.tensor.matmul(
                    wp[:w], lhsT=buf[:, 0:w], rhs=buf[:, 0:2], start=True, stop=True
                )

            # load input activations (single DMA: single completion sem)
            nc.sync.dma_start(
                slotA.rearrange("q (k t) -> q k t", k=HB),
                xt_d.rearrange("k q t -> q k t"),
            )
            warm(slotA)

            src, dst = slotA, slotB
            for l in range(n_layers):
                # ---- per-layer weight/bias loads ----
                nc.sync.dma_start(wbuf[:], wxb_d[l])
                warm(wbuf)
                nc.sync.dma_start(whb_sb[:], whb_d[l])
                warm(whb_sb)
                nc.sync.dma_start(bias_sb[:], bb_d[l])
                nc.sync.dma_start(b1_sb[:], b1b_d[l])
                nc.sync.dma_start(b2_sb[:], b2b_d[l])
                # touch the bias tiles on their consumer engines so later
                # instructions carry at most one unobserved sem wait
                # (several ISA structs have a single wait slot).
                tch = tmp.tile([P, 1], FP32, tag="touch")
                nc.vector.tensor_copy(out=tch, in_=bias_sb[:, 0:1])
                nc.vector.tensor_copy(out=tch, in_=b2_sb[:, 0:1])
                tch2 = tmp.tile([P, 1], FP32, tag="touch2")
                nc.scalar.copy(out=tch2, in_=b1_sb[:, 0:1])

                # ---- xg = x @ Wx + b  (transposed, gate blocks in M-ORDER) ----
                for (noff, nsz) in NSL:
                    for p in range(MB):
                        pt = pp.tile([P, 512], FP32, tag="ppt")
                        for k in range(KC):
                            nc.tensor.matmul(
                                pt[:, :nsz],
                                lhsT=wbuf[:, (k * MB + p) * P : (k * MB + p + 1) * P],
                                rhs=src[:, k * NTOK + noff : k * NTOK + noff + nsz],
                                start=(k == 0),
                                stop=(k == KC - 1),
                            )
                        # scatter into arena: col = t*32 + p*2 + b
                        out_ap = arena.rearrange("q (t c) -> q t c", c=2 * MB)[
                            :, noff // 2 : (noff + nsz) // 2, 2 * p : 2 * p + 2
                        ]
                        nc.vector.tensor_scalar_add(
                            out=out_ap,
                            in0=pt[:, :nsz].rearrange("q (t c) -> q t c", c=2),
                            scalar1=bias_sb[:, p : p + 1],
                        )

                # ---- LSTM scan over time (fully unrolled, static offsets) ----
                nc.vector.memset(cps, 0.0)
                nc.vector.memset(hT16, 0.0)
                h_view = h_all.rearrange("q (k t) -> q k t", k=HB)
                flip = l % 2 == 1
                for s in range(T_steps):
                    t = (T_steps - 1 - s) if flip else s  # data time index
                    xg_t = arena[:, t * 2 * MB : (t + 1) * 2 * MB]
                    # Two separate PSUM tiles so the i/f/g chain's deps clear
                    # after 48 matmuls while the o matmuls still run (Tile
                    # tracks dependencies per tile, not per column range).
                    gps = ps.tile([P, 24], FP32, tag="gps")
                    gpo = ps.tile([P, 8], FP32, tag="gpo")
                    for p in range(MB):
                        gdst = gps[:, 2 * p : 2 * p + 2] if p < 12 else gpo[
                            :, 2 * (p - 12) : 2 * (p - 12) + 2
                        ]
                        for k in range(KC):
                            nc.tensor.matmul(
                                gdst,
                                lhsT=whb_sb[
                                    :, (k * MB + p) * P : (k * MB + p + 1) * P
                                ],
                                rhs=hT16[:, 2 * k : 2 * k + 2],
                                start=(k == 0),
                                stop=(k == KC - 1),
                            )
                    gt = tmp.tile([P, 2 * MB], FP32, tag="gt")
                    # stage 1: i/f/g math (runs while the o matmuls finish)
                    nc.vector.tensor_add(out=gps, in0=gps, in1=xg_t[:, 0:24])
                    nc.scalar.activation(
                        out=gt[:, 0:16],
                        in_=gps[:, 0:16],
                        func=mybir.ActivationFunctionType.Sigmoid,
                    )
                    nc.scalar.activation(
                        out=gt[:, SC_G],
                        in_=gps[:, SC_G],
                        func=mybir.ActivationFunctionType.Tanh,
                    )
                    t1 = tmp.tile([P, 2 * HB], FP32, tag="t1")
                    t2 = tmp.tile([P, 2 * HB], FP32, tag="t2")
                    nc.vector.tensor_mul(out=t1, in0=gt[:, SC_F], in1=cps)
                    nc.vector.tensor_mul(out=t2, in0=gt[:, SC_I], in1=gt[:, SC_G])
                    nc.vector.tensor_add(out=cps, in0=t1, in1=t2)
                    th = tmp.tile([P, 2 * HB], FP32, tag="th")
                    nc.scalar.activation(
                        out=th, in_=cps, func=mybir.ActivationFunctionType.Tanh
                    )
                    # stage 2: o gate
                    nc.vector.tensor_add(out=gpo, in0=gpo, in1=xg_t[:, SC_O])
                    nc.scalar.activation(
                        out=gt[:, SC_O],
                        in_=gpo,
                        func=mybir.ActivationFunctionType.Sigmoid,
                    )
                    o3 = gt[:, SC_O].rearrange("q (k c) -> q k c", c=2)
                    th3 = th.rearrange("q (k c) -> q k c", c=2)
                    # hT16 first: it is the only thing the next step needs
                    nc.vector.tensor_mul(
                        out=hT16.rearrange("q (k c) -> q k c", c=2), in0=o3, in1=th3
                    )
                    nc.vector.tensor_mul(
                        out=h_view[:, :, 2 * t : 2 * t + 2], in0=o3, in1=th3
                    )

                # ---- FFN phase A: z = relu(h @ W1 + b1) ----
                nc.sync.dma_start(wbuf[:], w1b_d[l])
                warm(wbuf)
                for (noff, nsz) in NSL:
                    for p in range(FB):
                        pt = pp.tile([P, 512], FP32, tag="ppt")
                        for k in range(KC):
                            nc.tensor.matmul(
                                pt[:, :nsz],
                                lhsT=wbuf[:, (k * FB + p) * P : (k * FB + p + 1) * P],
                                rhs=h_all[:, k * NTOK + noff : k * NTOK + noff + nsz],
                                start=(k == 0),
                                stop=(k == KC - 1),
                            )
                        nc.scalar.activation(
                            out=arena[:, p * NTOK + noff : p * NTOK + noff + nsz],
                            in_=pt[:, :nsz],
                            func=mybir.ActivationFunctionType.Relu,
                            bias=b1_sb[:, p : p + 1],
                        )

                # ---- FFN phase B: y = z @ W2 + b2 ----
                nc.sync.dma_start(wbuf[:], w2b_d[l])
                warm(wbuf)
                for (noff, nsz) in NSL:
                    for m in range(HB):
                        pt = pp.tile([P, 512], FP32, tag="ppt")
                        for k in range(FB):
                            nc.tensor.matmul(
                                pt[:, :nsz],
                                lhsT=wbuf[:, (k * HB + m) * P : (k * HB + m + 1) * P],
                                rhs=arena[:, k * NTOK + noff : k * NTOK + noff + nsz],
                                start=(k == 0),
                                stop=(k == FB - 1),
                            )
                        nc.vector.tensor_scalar_add(
                            out=dst[:, m * NTOK + noff : m * NTOK + noff + nsz],
                            in0=pt[:, :nsz],
                            scalar1=b2_sb[:, m : m + 1],
                        )

                src, dst = dst, src

            for r in range(HB):
                nc.sync.dma_start(out_d[r], src[:, r * NTOK : (r + 1) * NTOK])

    nc.compile()
    return nc


# ---------------- host-side data prep ----------------


def _prep_gate_blocks(W: np.ndarray, dtype, reorder: bool) -> np.ndarray:
    """[K*P, M*P] -> [P, KC*Mblocks*P] block layout for stationary lhsT use.

    Row r of the result holds, for block (k, p), the weight W[k*P + r, col]
    at free position (k*Mb + p)*P + c.  `reorder` applies the [i,f,o,g]
    gate M-ORDER permutation.
    """
    KP, MP = W.shape
    kc, mb = KP // P, MP // P
    v = W.reshape(kc, P, mb, P)
    if reorder:
        v = v[:, :, ORIG_BLOCK, :]
    return np.ascontiguousarray(v.transpose(1, 0, 2, 3).reshape(P, kc * mb * P)).astype(
        dtype
    )


def _prep_bias(b: np.ndarray, reorder: bool) -> np.ndarray:
    """[M*P] -> [P, Mblocks] per-partition bias columns."""
    mb = b.shape[0] // P
    v = b.reshape(mb, P)
    if reorder:
        v = v[ORIG_BLOCK]
    return np.ascontiguousarray(v.T).astype(np.float32)


def prep_weights(Wx, Wh, b, W1, b1, W2, b2, n_layers):
    whb = np.stack([_prep_gate_blocks(Wh[l], ml_dtypes.bfloat16, True) for l in range(n_layers)])
    wxb = np.stack([_prep_gate_blocks(Wx[l], np.float32, True) for l in range(n_layers)])
    w1b = np.stack([_prep_gate_blocks(W1[l], np.float32, False) for l in range(n_layers)])
    w2b = np.stack([_prep_gate_blocks(W2[l], np.float32, False) for l in range(n_layers)])
    bb = np.stack([_prep_bias(b[l], True) for l in range(n_layers)])
    b1b = np.stack([_prep_bias(b1[l], False) for l in range(n_layers)])
    b2b = np.stack([_prep_bias(b2[l], False) for l in range(n_layers)])
    return dict(whb=whb, wxb=wxb, w1b=w1b, w2b=w2b, bb=bb, b1b=b1b, b2b=b2b)


def prep_x_core(x_c: np.ndarray) -> np.ndarray:
    """[BL, T, H] -> [HB, P, BL*T] transposed, tokens (t*2+b)-interleaved."""
    bl, t, h = x_c.shape
    v = x_c.transpose(2, 1, 0).reshape(h, t * bl)  # [H, T*BL] t-major b-minor
    return np.ascontiguousarray(v.reshape(HB, P, t * bl)).astype(np.float32)


def unprep_out_core(o: np.ndarray, t_steps: int) -> np.ndarray:
    """[HB, P, BL*T] -> [BL, T, H]."""
    v = o.reshape(H, t_steps, BL)
    return np.ascontiguousarray(v.transpose(2, 1, 0))


_NC_CACHE = {}


def run_cores(inputs: dict, t_steps=T, n_layers=L, unroll=8, trace=False):
    """Shard inputs, run the SPMD kernel on all 8 cores, return per-core
    outputs plus the raw BassKernelResults (for profiling)."""
    from concourse.bass_utils import run_bass_kernel_spmd

    x = np.asarray(inputs["x"], np.float32)
    wd = prep_weights(
        np.asarray(inputs["Wx"], np.float32),
        np.asarray(inputs["Wh"], np.float32),
        np.asarray(inputs["b"], np.float32),
        np.asarray(inputs["W1"], np.float32),
        np.asarray(inputs["b1"], np.float32),
        np.asarray(inputs["W2"], np.float32),
        np.asarray(inputs["b2"], np.float32),
        n_layers,
    )
    in_maps = []
    for c in range(NCORES):
        m = dict(wd)
        m["xt"] = prep_x_core(x[c * BL : (c + 1) * BL])
        in_maps.append(m)

    key = (t_steps, n_layers)
    if key not in _NC_CACHE:
        _NC_CACHE[key] = _build_nc(t_steps, n_layers, unroll)
    nc = _NC_CACHE[key]
    res = run_bass_kernel_spmd(nc, in_maps, core_ids=list(range(NCORES)), trace=trace)
    outs = [unprep_out_core(res.results[c]["out"], t_steps) for c in range(NCORES)]
    return np.concatenate(outs, axis=0), res


def kernel(**inputs) -> np.ndarray:
    out, _ = run_cores(inputs)
    return out.astype(np.float32)



# revision 8
# speedup vs baseline: 1.1488x; 1.1488x over previous
"""Trainium2 Bass kernel: 4-layer alternating-direction LSTM encoder with
per-layer FFN.  Data-parallel over batch across 8 NeuronCores.

v2 design notes (all-bf16 + chunk-staggered scan):
 - Everything "transposed": feature dim on partitions, tokens interleaved
   (t*2 + b) on the free axis, b in {0,1} local batch.
 - All weights/activations bf16 (fp32 PSUM accumulation); rel err ~0.7%.
 - Gate blocks are ordered chunk-major [i_j f_j o_j g_j] for hidden chunk
   j in {0..3}: block position m = j*4 + g maps to natural block
   perm[m] = [j, 4+j, 12+j, 8+j][g].  Each chunk owns one PSUM tile
   [128,8] with cols [i(2) f(2) o(2) g(2)] so sigmoid covers cols 0:6 and
   tanh cols 6:8 in single ACT ops.
 - The recurrent step never issues start=True matmuls: xg_t is DVE-copied
   into the PSUM tile beforehand (has_written bits are set once by init
   dummies), and all 16 matmuls per chunk accumulate onto it.  This both
   removes the xg adds from the critical chain and makes the accumulation
   order-independent.
 - Chunk j's matmuls are emitted as pairs 16j..16j+15 of the step; its
   tail produces h-chunk j (hT[j]) ~2.1us later, just in time for step
   t+1's k=j matmuls (pair 16j) -- a software-pipelined stagger that
   hides most of the serial gate-math chain behind PE work.
 - Sequence flips (odd layers) handled by negated time indexing only.
"""

import os
import sys

sys.path.insert(0, "/opt/trn_rl_repo")

import numpy as np
import ml_dtypes

import concourse.bass as bass
import concourse.bacc as bacc
import concourse.tile as tile
from concourse import mybir
from concourse.bass import ds

FP32 = mybir.dt.float32
BF16 = mybir.dt.bfloat16

L, H, F = 4, 512, 2048
B, T = 16, 512
NCORES = 8
BL = B // NCORES  # local batch per core
P = 128
KC = H // P  # 4 contraction chunks of H
MB = 4 * H // P  # 16 gate blocks
FB = F // P  # 16 filter blocks
HB = H // P  # 4 hidden blocks
NCH = 4  # hidden chunks in the scan
# chunk-major gate order: position m = j*4 + g, g in [i, f, o, g]
PERM = [g4 + j if g4 != 12 else 12 + j for j in range(4) for g4 in (0, 4, 12, 8)]
# PERM[j*4+0]=j (i), [..+1]=4+j (f), [..+2]=12+j (o), [..+3]=8+j (g)

SIG = mybir.ActivationFunctionType.Sigmoid
TANH = mybir.ActivationFunctionType.Tanh
RELU = mybir.ActivationFunctionType.Relu


def _build_nc(T_steps: int, n_layers: int) -> bass.Bass:
    """Build the per-core Bass program (identical on all cores)."""
    NTOK = BL * T_steps
    NSL = []
    off = 0
    while off < NTOK:
        sz = min(512, NTOK - off)
        NSL.append((off, sz))
        off += sz

    nc = bacc.Bacc()

    xt_d = nc.dram_tensor("xt", [HB, P, NTOK], BF16, kind="ExternalInput")
    whb_d = nc.dram_tensor("whb", [n_layers, P, KC * MB * P], BF16, kind="ExternalInput")
    wxb_d = nc.dram_tensor("wxb", [n_layers, P, KC * MB * P], BF16, kind="ExternalInput")
    w1b_d = nc.dram_tensor("w1b", [n_layers, P, KC * FB * P], BF16, kind="ExternalInput")
    w2b_d = nc.dram_tensor("w2b", [n_layers, P, FB * HB * P], BF16, kind="ExternalInput")
    bb_d = nc.dram_tensor("bb", [n_layers, P, MB], FP32, kind="ExternalInput")
    b1b_d = nc.dram_tensor("b1b", [n_layers, P, FB], FP32, kind="ExternalInput")
    b2b_d = nc.dram_tensor("b2b", [n_layers, P, HB], FP32, kind="ExternalInput")
    out_d = nc.dram_tensor("out", [HB, P, NTOK], BF16, kind="ExternalOutput")

    with tile.TileContext(nc) as tc:
        with (
            tc.tile_pool(name="state", bufs=1) as state,
            tc.tile_pool(name="tmp", bufs=3) as tmp,
            tc.tile_pool(name="psumG", bufs=2, space="PSUM") as pp,
            tc.tile_pool(name="psumS", bufs=1, space="PSUM") as ps,
        ):
            slotA = state.tile([P, HB * NTOK], BF16, tag="slotA")
            slotB = state.tile([P, HB * NTOK], BF16, tag="slotB")
            arena = state.tile([P, T_steps * 2 * MB], BF16, tag="arena")  # xg | z
            h_all = state.tile([P, HB * NTOK], BF16, tag="h_all")
            wbuf = state.tile([P, KC * MB * P], BF16, tag="wbuf")
            whb_sb = state.tile([P, KC * MB * P], BF16, tag="whb_sb")
            bias_sb = state.tile([P, MB], FP32, tag="bias_sb")
            b1_sb = state.tile([P, FB], FP32, tag="b1_sb")
            b2_sb = state.tile([P, HB], FP32, tag="b2_sb")
            # per-chunk recurrent state: h chunks (bf16, matmul rhs) and c
            hT = [
                state.tile([P, 2], BF16, tag=f"hT{j}", name=f"hT{j}")
                for j in range(NCH)
            ]
            cst = [
                state.tile([P, 2], FP32, tag=f"c{j}", name=f"c{j}")
                for j in range(NCH)
            ]
            # per-chunk gate PSUM tiles [i f o g] x 2 batch cols
            gp = [
                ps.tile([P, 8], FP32, tag=f"gp{j}", bufs=1, name=f"gp{j}")
                for j in range(NCH)
            ]

            def warm(buf):
                # Dummy matmul reading only `buf`: makes the PE observe the
                # buffer's DMA semaphore so real matmuls don't accumulate
                # more sync-waits than the LDWEIGHTS ISA slot budget.
                wp = ps.tile([P, 2], FP32, tag="warmps", bufs=1)
                w = min(buf.shape[1], P)
                nc.tensor.matmul(
                    wp[:w], lhsT=buf[:, 0:w], rhs=buf[:, 0:2], start=True, stop=True
                )

            # load input activations (single DMA: single completion sem)
            nc.sync.dma_start(
                slotA.rearrange("q (k t) -> q k t", k=HB),
                xt_d.rearrange("k q t -> q k t"),
            )
            warm(slotA)

            src, dst = slotA, slotB
            for l in range(n_layers):
                # ---- per-layer weight/bias loads ----
                nc.sync.dma_start(wbuf[:], wxb_d[l])
                warm(wbuf)
                nc.sync.dma_start(whb_sb[:], whb_d[l])
                warm(whb_sb)
                nc.sync.dma_start(bias_sb[:], bb_d[l])
                nc.sync.dma_start(b1_sb[:], b1b_d[l])
                nc.sync.dma_start(b2_sb[:], b2b_d[l])
                if l == 0:
                    # Initialize the scan PSUM tiles' has_written bits: a
                    # start=True dummy covers the tile (clearing+rewriting
                    # its bank's bits), then a start=False dummy re-marks
                    # every byte written so the steady-state flags=0
                    # matmuls always accumulate.
                    for j in range(NCH):
                        nc.tensor.matmul(
                            gp[j], lhsT=wbuf[:, 0:P], rhs=wbuf[:, 0:8],
                            start=True, stop=True, skip_group_check=True,
                        )
                    for j in range(NCH):
                        nc.tensor.matmul(
                            gp[j], lhsT=wbuf[:, 0:P], rhs=wbuf[:, 0:8],
                            start=False, stop=True, skip_group_check=True,
                        )
                tch = tmp.tile([P, 1], FP32, tag="touch")
                nc.vector.tensor_copy(out=tch, in_=bias_sb[:, 0:1])
                nc.vector.tensor_copy(out=tch, in_=b2_sb[:, 0:1])
                tch2 = tmp.tile([P, 1], FP32, tag="touch2")
                nc.scalar.copy(out=tch2, in_=b1_sb[:, 0:1])

                # ---- xg = x @ Wx + b  (gate blocks in chunk-major order) ----
                for (noff, nsz) in NSL:
                    for p in range(MB):
                        pt = pp.tile([P, 512], FP32, tag="ppt")
                        for k in range(KC):
                            nc.tensor.matmul(
                                pt[:, :nsz],
                                lhsT=wbuf[:, (k * MB + p) * P : (k * MB + p + 1) * P],
                                rhs=src[:, k * NTOK + noff : k * NTOK + noff + nsz],
                                start=(k == 0),
                                stop=(k == KC - 1),
                            )
                        # scatter into arena: col = t*32 + p*2 + b
                        out_ap = arena.rearrange("q (t c) -> q t c", c=2 * MB)[
                            :, noff // 2 : (noff + nsz) // 2, 2 * p : 2 * p + 2
                        ]
                        nc.vector.tensor_scalar_add(
                            out=out_ap,
                            in0=pt[:, :nsz].rearrange("q (t c) -> q t c", c=2),
                            scalar1=bias_sb[:, p : p + 1],
                        )

                # ---- LSTM scan over time (chunk-staggered software pipeline) ----
                flip = l % 2 == 1
                for j in range(NCH):
                    nc.vector.memset(cst[j], 0.0)
                    nc.vector.memset(hT[j], 0.0)
                h_view = h_all.rearrange("q (k t) -> q k t", k=HB)
                t0 = (T_steps - 1) if flip else 0
                # preload xg of step 0 into the gate PSUM tiles
                for j in range(NCH):
                    nc.vector.tensor_copy(
                        out=gp[j], in_=arena[:, t0 * 2 * MB + 8 * j : t0 * 2 * MB + 8 * j + 8]
                    )

                for s in range(T_steps):
                    t = (T_steps - 1 - s) if flip else s  # data time index
                    tn = (T_steps - 2 - s) if flip else (s + 1)
                    last = s == T_steps - 1
                    # PE: 16 accumulate-matmuls per chunk, chunks in order
                    for j in range(NCH):
                        for g in range(4):
                            m = j * 4 + g
                            for k in range(KC):
                                nc.tensor.matmul(
                                    gp[j][:, 2 * g : 2 * g + 2],
                                    lhsT=whb_sb[:, (k * MB + m) * P : (k * MB + m + 1) * P],
                                    rhs=hT[k],
                                    start=False,
                                    stop=(k == KC - 1),
                                    skip_group_check=True,
                                )
                    # per-chunk gate tails (staggered; chunk j feeds next
                    # step's k=j matmuls)
                    gt = [
                        tmp.tile([P, 8], FP32, tag=f"gt{j}", name=f"gt{j}")
                        for j in range(NCH)
                    ]
                    th = [
                        tmp.tile([P, 2], FP32, tag=f"th{j}", name=f"th{j}")
                        for j in range(NCH)
                    ]
                    for j in range(NCH):
                        # gates: cols [i i f f o o g g]
                        nc.scalar.activation(out=gt[j][:, 0:6], in_=gp[j][:, 0:6], func=SIG)
                        nc.scalar.activation(out=gt[j][:, 6:8], in_=gp[j][:, 6:8], func=TANH)
                        t2 = tmp.tile([P, 2], FP32, tag=f"t2{j}", name=f"t2{j}")
                        nc.vector.tensor_mul(out=t2, in0=gt[j][:, 0:2], in1=gt[j][:, 6:8])
                        nc.vector.tensor_mul(out=cst[j], in0=cst[j], in1=gt[j][:, 2:4])
                        nc.vector.tensor_add(out=cst[j], in0=cst[j], in1=t2)
                        nc.scalar.activation(out=th[j], in_=cst[j], func=TANH)
                        # h chunk j (bf16): the only thing step t+1 needs
                        nc.vector.tensor_mul(out=hT[j], in0=gt[j][:, 4:6], in1=th[j])
                        # refill xg for the next step
                        if not last:
                            nc.vector.tensor_copy(
                                out=gp[j],
                                in_=arena[:, tn * 2 * MB + 8 * j : tn * 2 * MB + 8 * j + 8],
                            )
                    # h_all writes (off the critical chain; Pool is idle
                    # during the scan and ACT/DVE are near their op-rate
                    # budgets)
                    for j in range(NCH):
                        nc.gpsimd.tensor_copy(
                            out=h_view[:, j, 2 * t : 2 * t + 2], in_=hT[j]
                        )

                # ---- FFN phase A: z = relu(h @ W1 + b1) ----
                nc.sync.dma_start(wbuf[:], w1b_d[l])
                warm(wbuf)
                for (noff, nsz) in NSL:
                    for p in range(FB):
                        pt = pp.tile([P, 512], FP32, tag="ppt")
                        for k in range(KC):
                            nc.tensor.matmul(
                                pt[:, :nsz],
                                lhsT=wbuf[:, (k * FB + p) * P : (k * FB + p + 1) * P],
                                rhs=h_all[:, k * NTOK + noff : k * NTOK + noff + nsz],
                                start=(k == 0),
                                stop=(k == KC - 1),
                            )
                        nc.scalar.activation(
                            out=arena[:, p * NTOK + noff : p * NTOK + noff + nsz],
                            in_=pt[:, :nsz],
                            func=RELU,
                            bias=b1_sb[:, p : p + 1],
                        )

                # ---- FFN phase B: y = z @ W2 + b2 ----
                nc.sync.dma_start(wbuf[:], w2b_d[l])
                warm(wbuf)
                for (noff, nsz) in NSL:
                    for m in range(HB):
                        pt = pp.tile([P, 512], FP32, tag="ppt")
                        for k in range(FB):
                            nc.tensor.matmul(
                                pt[:, :nsz],
                                lhsT=wbuf[:, (k * HB + m) * P : (k * HB + m + 1) * P],
                                rhs=arena[:, k * NTOK + noff : k * NTOK + noff + nsz],
                                start=(k == 0),
                                stop=(k == FB - 1),
                            )
                        nc.vector.tensor_scalar_add(
                            out=dst[:, m * NTOK + noff : m * NTOK + noff + nsz],
                            in0=pt[:, :nsz],
                            scalar1=b2_sb[:, m : m + 1],
                        )

                src, dst = dst, src

            for r in range(HB):
                nc.sync.dma_start(out_d[r], src[:, r * NTOK : (r + 1) * NTOK])

    nc.compile()
    return nc


# ---------------- host-side data prep ----------------


def _prep_gate_blocks(W: np.ndarray, reorder: bool) -> np.ndarray:
    """[K*P, M*P] -> [P, KC*Mblocks*P] block layout for stationary lhsT use.

    Row r of the result holds, for block (k, p), the weight W[k*P + r, col]
    at free position (k*Mb + p)*P + c.  `reorder` applies the chunk-major
    [i f o g] gate permutation.
    """
    KP, MP = W.shape
    kc, mb = KP // P, MP // P
    v = W.reshape(kc, P, mb, P)
    if reorder:
        v = v[:, :, PERM, :]
    return np.ascontiguousarray(v.transpose(1, 0, 2, 3).reshape(P, kc * mb * P)).astype(
        ml_dtypes.bfloat16
    )


def _prep_bias(b: np.ndarray, reorder: bool) -> np.ndarray:
    """[M*P] -> [P, Mblocks] per-partition bias columns."""
    mb = b.shape[0] // P
    v = b.reshape(mb, P)
    if reorder:
        v = v[PERM]
    return np.ascontiguousarray(v.T).astype(np.float32)


def prep_weights(Wx, Wh, b, W1, b1, W2, b2, n_layers):
    whb = np.stack([_prep_gate_blocks(Wh[l], True) for l in range(n_layers)])
    wxb = np.stack([_prep_gate_blocks(Wx[l], True) for l in range(n_layers)])
    w1b = np.stack([_prep_gate_blocks(W1[l], False) for l in range(n_layers)])
    w2b = np.stack([_prep_gate_blocks(W2[l], False) for l in range(n_layers)])
    bb = np.stack([_prep_bias(b[l], True) for l in range(n_layers)])
    b1b = np.stack([_prep_bias(b1[l], False) for l in range(n_layers)])
    b2b = np.stack([_prep_bias(b2[l], False) for l in range(n_layers)])
    return dict(whb=whb, wxb=wxb, w1b=w1b, w2b=w2b, bb=bb, b1b=b1b, b2b=b2b)


def prep_x_core(x_c: np.ndarray) -> np.ndarray:
    """[BL, T, H] -> [HB, P, BL*T] transposed, tokens (t*2+b)-interleaved."""
    bl, t, h = x_c.shape
    v = x_c.transpose(2, 1, 0).reshape(h, t * bl)  # [H, T*BL] t-major b-minor
    return np.ascontiguousarray(v.reshape(HB, P, t * bl)).astype(ml_dtypes.bfloat16)


def unprep_out_core(o: np.ndarray, t_steps: int) -> np.ndarray:
    """[HB, P, BL*T] -> [BL, T, H]."""
    v = np.asarray(o, dtype=np.float32).reshape(H, t_steps, BL)
    return np.ascontiguousarray(v.transpose(2, 1, 0))


_NC_CACHE = {}


def run_cores(inputs: dict, t_steps=T, n_layers=L, trace=False):
    """Shard inputs, run the SPMD kernel on all 8 cores, return per-core
    outputs plus the raw BassKernelResults (for profiling)."""
    from concourse.bass_utils import run_bass_kernel_spmd

    x = np.asarray(inputs["x"], np.float32)
    wd = prep_weights(
        np.asarray(inputs["Wx"], np.float32),
        np.asarray(inputs["Wh"], np.float32),
        np.asarray(inputs["b"], np.float32),
        np.asarray(inputs["W1"], np.float32),
        np.asarray(inputs["b1"], np.float32),
        np.asarray(inputs["W2"], np.float32),
        np.asarray(inputs["b2"], np.float32),
        n_layers,
    )
    in_maps = []
    for c in range(NCORES):
        m = dict(wd)
        m["xt"] = prep_x_core(x[c * BL : (c + 1) * BL])
        in_maps.append(m)

    key = (t_steps, n_layers)
    if key not in _NC_CACHE:
        _NC_CACHE[key] = _build_nc(t_steps, n_layers)
    nc = _NC_CACHE[key]
    res = run_bass_kernel_spmd(nc, in_maps, core_ids=list(range(NCORES)), trace=trace)
    outs = [unprep_out_core(res.results[c]["out"], t_steps) for c in range(NCORES)]
    return np.concatenate(outs, axis=0), res


def kernel(**inputs) -> np.ndarray:
    out, _ = run_cores(inputs)
    return out.astype(np.float32)


# revision 12
# speedup vs baseline: 1.3570x; 1.1812x over previous
"""Trainium2 Bass kernel: 4-layer alternating-direction LSTM encoder with
per-layer FFN.  Data-parallel over batch across 8 NeuronCores.

v2 design notes (all-bf16 + chunk-staggered scan):
 - Everything "transposed": feature dim on partitions, tokens interleaved
   (t*2 + b) on the free axis, b in {0,1} local batch.
 - All weights/activations bf16 (fp32 PSUM accumulation); rel err ~0.7%.
 - Gate blocks are ordered chunk-major [i_j f_j o_j g_j] for hidden chunk
   j in {0..3}: block position m = j*4 + g maps to natural block
   perm[m] = [j, 4+j, 12+j, 8+j][g].  Each chunk owns one PSUM tile
   [128,8] with cols [i(2) f(2) o(2) g(2)] so sigmoid covers cols 0:6 and
   tanh cols 6:8 in single ACT ops.
 - The recurrent step never issues start=True matmuls: xg_t is DVE-copied
   into the PSUM tile beforehand (has_written bits are set once by init
   dummies), and all 16 matmuls per chunk accumulate onto it.  This both
   removes the xg adds from the critical chain and makes the accumulation
   order-independent.
 - Chunk j's matmuls are emitted as pairs 16j..16j+15 of the step; its
   tail produces h-chunk j (hT[j]) ~2.1us later, just in time for step
   t+1's k=j matmuls (pair 16j) -- a software-pipelined stagger that
   hides most of the serial gate-math chain behind PE work.
 - Sequence flips (odd layers) handled by negated time indexing only.
"""

import os
import sys

sys.path.insert(0, "/opt/trn_rl_repo")

import numpy as np
import ml_dtypes

import concourse.bass as bass
import concourse.bacc as bacc
import concourse.tile as tile
from concourse import mybir
from concourse.bass import ds

FP32 = mybir.dt.float32
BF16 = mybir.dt.bfloat16

L, H, F = 4, 512, 2048
B, T = 16, 512
NCORES = 8
BL = B // NCORES  # local batch per core
P = 128
KC = H // P  # 4 contraction chunks of H
MB = 4 * H // P  # 16 gate blocks
FB = F // P  # 16 filter blocks
HB = H // P  # 4 hidden blocks
NCH = 4  # hidden chunks in the scan
# chunk-major gate order: position m = j*4 + g, g in [i, f, o, g]
PERM = [g4 + j if g4 != 12 else 12 + j for j in range(4) for g4 in (0, 4, 12, 8)]
# PERM[j*4+0]=j (i), [..+1]=4+j (f), [..+2]=12+j (o), [..+3]=8+j (g)

SIG = mybir.ActivationFunctionType.Sigmoid
TANH = mybir.ActivationFunctionType.Tanh
RELU = mybir.ActivationFunctionType.Relu


def _build_nc(T_steps: int, n_layers: int) -> bass.Bass:
    """Build the per-core Bass program (identical on all cores)."""
    NTOK = BL * T_steps
    NSL = []
    off = 0
    while off < NTOK:
        sz = min(512, NTOK - off)
        NSL.append((off, sz))
        off += sz

    nc = bacc.Bacc()

    xt_d = nc.dram_tensor("xt", [HB, P, NTOK], BF16, kind="ExternalInput")
    whb_d = nc.dram_tensor("whb", [n_layers, P, KC * MB * P], BF16, kind="ExternalInput")
    wxb_d = nc.dram_tensor("wxb", [n_layers, P, KC * MB * P], BF16, kind="ExternalInput")
    w1b_d = nc.dram_tensor("w1b", [n_layers, P, KC * FB * P], BF16, kind="ExternalInput")
    w2b_d = nc.dram_tensor("w2b", [n_layers, P, FB * HB * P], BF16, kind="ExternalInput")
    bb_d = nc.dram_tensor("bb", [n_layers, P, MB], FP32, kind="ExternalInput")
    b1b_d = nc.dram_tensor("b1b", [n_layers, P, FB], FP32, kind="ExternalInput")
    b2b_d = nc.dram_tensor("b2b", [n_layers, P, HB], FP32, kind="ExternalInput")
    out_d = nc.dram_tensor("out", [HB, P, NTOK], BF16, kind="ExternalOutput")

    with tile.TileContext(nc) as tc:
        with (
            tc.tile_pool(name="state", bufs=1) as state,
            tc.tile_pool(name="tmp", bufs=3) as tmp,
            tc.tile_pool(name="psumG", bufs=2, space="PSUM") as pp,
            tc.tile_pool(name="psumS", bufs=1, space="PSUM") as ps,
        ):
            slotA = state.tile([P, HB * NTOK], BF16, tag="slotA")
            slotB = state.tile([P, HB * NTOK], BF16, tag="slotB")
            arena = state.tile([P, T_steps * 2 * MB], BF16, tag="arena")  # xg | z
            h_all = state.tile([P, HB * NTOK], BF16, tag="h_all")
            wbuf = state.tile([P, KC * MB * P], BF16, tag="wbuf")
            whb_sb = state.tile([P, KC * MB * P], BF16, tag="whb_sb")
            bias_sb = state.tile([P, MB], FP32, tag="bias_sb")
            b1_sb = state.tile([P, FB], FP32, tag="b1_sb")
            b2_sb = state.tile([P, HB], FP32, tag="b2_sb")
            # per-chunk recurrent state: h chunks (bf16, matmul rhs) and c
            hT = [
                state.tile([P, 2], BF16, tag=f"hT{j}", name=f"hT{j}")
                for j in range(NCH)
            ]
            cst = [
                state.tile([P, 2], FP32, tag=f"c{j}", name=f"c{j}")
                for j in range(NCH)
            ]
            # per-chunk gate PSUM tiles [i f o g] x 2 batch cols
            gp = [
                ps.tile([P, 8], FP32, tag=f"gp{j}", bufs=1, name=f"gp{j}")
                for j in range(NCH)
            ]

            def warm(buf):
                # Dummy matmul reading only `buf`: makes the PE observe the
                # buffer's DMA semaphore so real matmuls don't accumulate
                # more sync-waits than the LDWEIGHTS ISA slot budget.
                wp = ps.tile([P, 2], FP32, tag="warmps", bufs=1)
                w = min(buf.shape[1], P)
                nc.tensor.matmul(
                    wp[:w], lhsT=buf[:, 0:w], rhs=buf[:, 0:2], start=True, stop=True
                )

            # load input activations (single DMA: single completion sem)
            nc.sync.dma_start(
                slotA.rearrange("q (k t) -> q k t", k=HB),
                xt_d.rearrange("k q t -> q k t"),
            )
            warm(slotA)

            src, dst = slotA, slotB
            for l in range(n_layers):
                # ---- per-layer weight/bias loads ----
                nc.sync.dma_start(wbuf[:], wxb_d[l])
                warm(wbuf)
                nc.sync.dma_start(whb_sb[:], whb_d[l])
                warm(whb_sb)
                nc.sync.dma_start(bias_sb[:], bb_d[l])
                nc.sync.dma_start(b1_sb[:], b1b_d[l])
                nc.sync.dma_start(b2_sb[:], b2b_d[l])
                if l == 0:
                    # Initialize the scan PSUM tiles' has_written bits: a
                    # start=True dummy covers the tile (clearing+rewriting
                    # its bank's bits), then a start=False dummy re-marks
                    # every byte written so the steady-state flags=0
                    # matmuls always accumulate.
                    for j in range(NCH):
                        nc.tensor.matmul(
                            gp[j], lhsT=wbuf[:, 0:P], rhs=wbuf[:, 0:8],
                            start=True, stop=True, skip_group_check=True,
                        )
                    for j in range(NCH):
                        nc.tensor.matmul(
                            gp[j], lhsT=wbuf[:, 0:P], rhs=wbuf[:, 0:8],
                            start=False, stop=True, skip_group_check=True,
                        )
                tch = tmp.tile([P, 1], FP32, tag="touch")
                nc.vector.tensor_copy(out=tch, in_=bias_sb[:, 0:1])
                nc.vector.tensor_copy(out=tch, in_=b2_sb[:, 0:1])
                tch2 = tmp.tile([P, 1], FP32, tag="touch2")
                nc.scalar.copy(out=tch2, in_=b1_sb[:, 0:1])

                # ---- xg = x @ Wx + b  (gate blocks in chunk-major order) ----
                for (noff, nsz) in NSL:
                    for p in range(MB):
                        pt = pp.tile([P, 512], FP32, tag="ppt")
                        for k in range(KC):
                            nc.tensor.matmul(
                                pt[:, :nsz],
                                lhsT=wbuf[:, (k * MB + p) * P : (k * MB + p + 1) * P],
                                rhs=src[:, k * NTOK + noff : k * NTOK + noff + nsz],
                                start=(k == 0),
                                stop=(k == KC - 1),
                            )
                        # scatter into arena: col = t*32 + p*2 + b
                        out_ap = arena.rearrange("q (t c) -> q t c", c=2 * MB)[
                            :, noff // 2 : (noff + nsz) // 2, 2 * p : 2 * p + 2
                        ]
                        nc.vector.tensor_scalar_add(
                            out=out_ap,
                            in0=pt[:, :nsz].rearrange("q (t c) -> q t c", c=2),
                            scalar1=bias_sb[:, p : p + 1],
                        )

                # ---- LSTM scan over time (chunk-staggered software pipeline) ----
                flip = l % 2 == 1
                for j in range(NCH):
                    nc.vector.memset(cst[j], 0.0)
                    nc.vector.memset(hT[j], 0.0)
                h_view = h_all.rearrange("q (k t) -> q k t", k=HB)
                t0 = (T_steps - 1) if flip else 0
                # preload xg of step 0 into the gate PSUM tiles
                for j in range(NCH):
                    nc.vector.tensor_copy(
                        out=gp[j], in_=arena[:, t0 * 2 * MB + 8 * j : t0 * 2 * MB + 8 * j + 8]
                    )

                for s in range(T_steps):
                    t = (T_steps - 1 - s) if flip else s  # data time index
                    tn = (T_steps - 2 - s) if flip else (s + 1)
                    last = s == T_steps - 1
                    # PE: 16 accumulate-matmuls per chunk, chunks in order
                    for j in range(NCH):
                        for g in range(4):
                            m = j * 4 + g
                            for k in range(KC):
                                nc.tensor.matmul(
                                    gp[j][:, 2 * g : 2 * g + 2],
                                    lhsT=whb_sb[:, (k * MB + m) * P : (k * MB + m + 1) * P],
                                    rhs=hT[k],
                                    start=False,
                                    stop=(k == KC - 1),
                                    skip_group_check=True,
                                )
                    # Per-chunk gate tails, software-pipelined.  Both ACT
                    # and DVE queues are strict FIFO, so ops are emitted in
                    # dependency-arrival order: a not-yet-ready op at the
                    # queue head would block later ready ops (head-of-line
                    # blocking).  gp[j] cols: [i i f f o o g g].
                    gt = [
                        tmp.tile([P, 8], FP32, tag=f"gt{j}", name=f"gt{j}")
                        for j in range(NCH)
                    ]
                    th = [
                        tmp.tile([P, 2], FP32, tag=f"th{j}", name=f"th{j}")
                        for j in range(NCH)
                    ]

                    def act_wave(j):
                        nc.scalar.activation(out=gt[j][:, 0:6], in_=gp[j][:, 0:6], func=SIG)
                        nc.scalar.activation(out=gt[j][:, 6:8], in_=gp[j][:, 6:8], func=TANH)

                    def h_out(j):
                        # h chunk j (bf16): the only thing step t+1 needs
                        nc.vector.tensor_mul(out=hT[j], in0=gt[j][:, 4:6], in1=th[j])
                        # refill xg for the next step
                        if not last:
                            nc.vector.tensor_copy(
                                out=gp[j],
                                in_=arena[:, tn * 2 * MB + 8 * j : tn * 2 * MB + 8 * j + 8],
                            )

                    act_wave(0)
                    act_wave(1)
                    for j in range(NCH):
                        # c_j = sig(f)*c_j + sig(i)*tanh(g)
                        t2 = tmp.tile([P, 2], FP32, tag=f"t2{j}", name=f"t2{j}")
                        nc.vector.tensor_mul(out=t2, in0=gt[j][:, 0:2], in1=gt[j][:, 6:8])
                        nc.vector.tensor_mul(out=cst[j], in0=cst[j], in1=gt[j][:, 2:4])
                        nc.vector.tensor_add(out=cst[j], in0=cst[j], in1=t2)
                        nc.scalar.activation(out=th[j], in_=cst[j], func=TANH)
                        if j + 2 < NCH:
                            act_wave(j + 2)
                        if j >= 1:
                            h_out(j - 1)
                    h_out(NCH - 1)
                    # h_all writes (off the critical chain; on DVE -- ACT is
                    # the op-rate-limiting engine in the scan)
                    for j in range(NCH):
                        nc.vector.tensor_copy(
                            out=h_view[:, j, 2 * t : 2 * t + 2], in_=hT[j]
                        )

                # ---- FFN phase A: z = relu(h @ W1 + b1) ----
                nc.sync.dma_start(wbuf[:], w1b_d[l])
                warm(wbuf)
                for (noff, nsz) in NSL:
                    for p in range(FB):
                        pt = pp.tile([P, 512], FP32, tag="ppt")
                        for k in range(KC):
                            nc.tensor.matmul(
                                pt[:, :nsz],
                                lhsT=wbuf[:, (k * FB + p) * P : (k * FB + p + 1) * P],
                                rhs=h_all[:, k * NTOK + noff : k * NTOK + noff + nsz],
                                start=(k == 0),
                                stop=(k == KC - 1),
                            )
                        nc.scalar.activation(
                            out=arena[:, p * NTOK + noff : p * NTOK + noff + nsz],
                            in_=pt[:, :nsz],
                            func=RELU,
                            bias=b1_sb[:, p : p + 1],
                        )

                # ---- FFN phase B: y = z @ W2 + b2 ----
                nc.sync.dma_start(wbuf[:], w2b_d[l])
                warm(wbuf)
                for (noff, nsz) in NSL:
                    for m in range(HB):
                        pt = pp.tile([P, 512], FP32, tag="ppt")
                        for k in range(FB):
                            nc.tensor.matmul(
                                pt[:, :nsz],
                                lhsT=wbuf[:, (k * HB + m) * P : (k * HB + m + 1) * P],
                                rhs=arena[:, k * NTOK + noff : k * NTOK + noff + nsz],
                                start=(k == 0),
                                stop=(k == FB - 1),
                            )
                        nc.vector.tensor_scalar_add(
                            out=dst[:, m * NTOK + noff : m * NTOK + noff + nsz],
                            in0=pt[:, :nsz],
                            scalar1=b2_sb[:, m : m + 1],
                        )

                src, dst = dst, src

            for r in range(HB):
                nc.sync.dma_start(out_d[r], src[:, r * NTOK : (r + 1) * NTOK])

    nc.compile()
    return nc


# ---------------- host-side data prep ----------------


def _prep_gate_blocks(W: np.ndarray, reorder: bool) -> np.ndarray:
    """[K*P, M*P] -> [P, KC*Mblocks*P] block layout for stationary lhsT use.

    Row r of the result holds, for block (k, p), the weight W[k*P + r, col]
    at free position (k*Mb + p)*P + c.  `reorder` applies the chunk-major
    [i f o g] gate permutation.
    """
    KP, MP = W.shape
    kc, mb = KP // P, MP // P
    v = W.reshape(kc, P, mb, P)
    if reorder:
        v = v[:, :, PERM, :]
    return np.ascontiguousarray(v.transpose(1, 0, 2, 3).reshape(P, kc * mb * P)).astype(
        ml_dtypes.bfloat16
    )


def _prep_bias(b: np.ndarray, reorder: bool) -> np.ndarray:
    """[M*P] -> [P, Mblocks] per-partition bias columns."""
    mb = b.shape[0] // P
    v = b.reshape(mb, P)
    if reorder:
        v = v[PERM]
    return np.ascontiguousarray(v.T).astype(np.float32)


def prep_weights(Wx, Wh, b, W1, b1, W2, b2, n_layers):
    whb = np.stack([_prep_gate_blocks(Wh[l], True) for l in range(n_layers)])
    wxb = np.stack([_prep_gate_blocks(Wx[l], True) for l in range(n_layers)])
    w1b = np.stack([_prep_gate_blocks(W1[l], False) for l in range(n_layers)])
    w2b = np.stack([_prep_gate_blocks(W2[l], False) for l in range(n_layers)])
    bb = np.stack([_prep_bias(b[l], True) for l in range(n_layers)])
    b1b = np.stack([_prep_bias(b1[l], False) for l in range(n_layers)])
    b2b = np.stack([_prep_bias(b2[l], False) for l in range(n_layers)])
    return dict(whb=whb, wxb=wxb, w1b=w1b, w2b=w2b, bb=bb, b1b=b1b, b2b=b2b)


def prep_x_core(x_c: np.ndarray) -> np.ndarray:
    """[BL, T, H] -> [HB, P, BL*T] transposed, tokens (t*2+b)-interleaved."""
    bl, t, h = x_c.shape
    v = x_c.transpose(2, 1, 0).reshape(h, t * bl)  # [H, T*BL] t-major b-minor
    return np.ascontiguousarray(v.reshape(HB, P, t * bl)).astype(ml_dtypes.bfloat16)


def unprep_out_core(o: np.ndarray, t_steps: int) -> np.ndarray:
    """[HB, P, BL*T] -> [BL, T, H]."""
    v = np.asarray(o, dtype=np.float32).reshape(H, t_steps, BL)
    return np.ascontiguousarray(v.transpose(2, 1, 0))


_NC_CACHE = {}


def run_cores(inputs: dict, t_steps=T, n_layers=L, trace=False):
    """Shard inputs, run the SPMD kernel on all 8 cores, return per-core
    outputs plus the raw BassKernelResults (for profiling)."""
    from concourse.bass_utils import run_bass_kernel_spmd

    x = np.asarray(inputs["x"], np.float32)
    wd = prep_weights(
        np.asarray(inputs["Wx"], np.float32),
        np.asarray(inputs["Wh"], np.float32),
        np.asarray(inputs["b"], np.float32),
        np.asarray(inputs["W1"], np.float32),
        np.asarray(inputs["b1"], np.float32),
        np.asarray(inputs["W2"], np.float32),
        np.asarray(inputs["b2"], np.float32),
        n_layers,
    )
    in_maps = []
    for c in range(NCORES):
        m = dict(wd)
        m["xt"] = prep_x_core(x[c * BL : (c + 1) * BL])
        in_maps.append(m)

    key = (t_steps, n_layers)
    if key not in _NC_CACHE:
        _NC_CACHE[key] = _build_nc(t_steps, n_layers)
    nc = _NC_CACHE[key]
    res = run_bass_kernel_spmd(nc, in_maps, core_ids=list(range(NCORES)), trace=trace)
    outs = [unprep_out_core(res.results[c]["out"], t_steps) for c in range(NCORES)]
    return np.concatenate(outs, axis=0), res


def kernel(**inputs) -> np.ndarray:
    out, _ = run_cores(inputs)
    return out.astype(np.float32)


# revision 22
# speedup vs baseline: 1.5370x; 1.1327x over previous
"""Trainium2 Bass kernel: 4-layer alternating-direction LSTM encoder with
per-layer FFN.  Data-parallel over batch across 8 NeuronCores.

v2 design notes (all-bf16 + chunk-staggered scan):
 - Everything "transposed": feature dim on partitions, tokens interleaved
   (t*2 + b) on the free axis, b in {0,1} local batch.
 - All weights/activations bf16 (fp32 PSUM accumulation); rel err ~0.7%.
 - Gate blocks are ordered chunk-major [i_j f_j o_j g_j] for hidden chunk
   j in {0..3}: block position m = j*4 + g maps to natural block
   perm[m] = [j, 4+j, 12+j, 8+j][g].  Each chunk owns one PSUM tile
   [128,8] with cols [i(2) f(2) o(2) g(2)] so sigmoid covers cols 0:6 and
   tanh cols 6:8 in single ACT ops.
 - The recurrent step never issues start=True matmuls: xg_t is DVE-copied
   into the PSUM tile beforehand (has_written bits are set once by init
   dummies), and all 16 matmuls per chunk accumulate onto it.  This both
   removes the xg adds from the critical chain and makes the accumulation
   order-independent.
 - Chunk j's matmuls are emitted as pairs 16j..16j+15 of the step; its
   tail produces h-chunk j (hT[j]) ~2.1us later, just in time for step
   t+1's k=j matmuls (pair 16j) -- a software-pipelined stagger that
   hides most of the serial gate-math chain behind PE work.
 - Sequence flips (odd layers) handled by negated time indexing only.
"""

import os
import sys

sys.path.insert(0, "/opt/trn_rl_repo")

import numpy as np
import ml_dtypes

import concourse.bass as bass
import concourse.bacc as bacc
import concourse.tile as tile
from concourse import mybir
from concourse.bass import ds

FP32 = mybir.dt.float32
BF16 = mybir.dt.bfloat16

L, H, F = 4, 512, 2048
B, T = 16, 512
NCORES = 8
BL = B // NCORES  # local batch per core
P = 128
KC = H // P  # 4 contraction chunks of H
MB = 4 * H // P  # 16 gate blocks
FB = F // P  # 16 filter blocks
HB = H // P  # 4 hidden blocks
NCH = 4  # hidden chunks in the scan
# chunk-major gate order: position m = j*4 + g, g in [i, f, o, g]
PERM = [g4 + j for j in range(4) for g4 in (0, 4, 12, 8)]
# PERM[j*4+0]=j (i), [..+1]=4+j (f), [..+2]=12+j (o), [..+3]=8+j (g)

SIG = mybir.ActivationFunctionType.Sigmoid
TANH = mybir.ActivationFunctionType.Tanh
RELU = mybir.ActivationFunctionType.Relu


def _build_nc(T_steps: int, n_layers: int) -> bass.Bass:
    """Build the per-core Bass program (identical on all cores)."""
    NTOK = BL * T_steps
    NSL = []
    off = 0
    while off < NTOK:
        sz = min(512, NTOK - off)
        NSL.append((off, sz))
        off += sz

    nc = bacc.Bacc()

    xt_d = nc.dram_tensor("xt", [HB, P, NTOK], BF16, kind="ExternalInput")
    whb_d = nc.dram_tensor("whb", [n_layers, P, KC * MB * P], BF16, kind="ExternalInput")
    wxb_d = nc.dram_tensor("wxb", [n_layers, P, KC * MB * P], BF16, kind="ExternalInput")
    w1b_d = nc.dram_tensor("w1b", [n_layers, P, KC * FB * P], BF16, kind="ExternalInput")
    w2b_d = nc.dram_tensor("w2b", [n_layers, P, FB * HB * P], BF16, kind="ExternalInput")
    bb_d = nc.dram_tensor("bb", [n_layers, P, MB], FP32, kind="ExternalInput")
    b1b_d = nc.dram_tensor("b1b", [n_layers, P, FB], FP32, kind="ExternalInput")
    b2b_d = nc.dram_tensor("b2b", [n_layers, P, HB], FP32, kind="ExternalInput")
    out_d = nc.dram_tensor("out", [HB, P, NTOK], BF16, kind="ExternalOutput")

    with tile.TileContext(nc) as tc:
        with (
            tc.tile_pool(name="state", bufs=1) as state,
            tc.tile_pool(name="tmp", bufs=3) as tmp,
            tc.tile_pool(name="psumG", bufs=2, space="PSUM") as pp,
            tc.tile_pool(name="psumS", bufs=1, space="PSUM") as ps,
        ):
            slotA = state.tile([P, HB * NTOK], BF16, tag="slotA")
            slotB = state.tile([P, HB * NTOK], BF16, tag="slotB")
            arena = state.tile([P, T_steps * 2 * MB], BF16, tag="arena")  # xg | z
            h_all = state.tile([P, HB * NTOK], BF16, tag="h_all")
            wbuf = state.tile([P, KC * MB * P], BF16, tag="wbuf")
            whb_sb = state.tile([P, KC * MB * P], BF16, tag="whb_sb")
            bias_sb = state.tile([P, MB], FP32, tag="bias_sb")
            b1_sb = state.tile([P, FB], FP32, tag="b1_sb")
            b2_sb = state.tile([P, HB], FP32, tag="b2_sb")
            # per-chunk recurrent state: h chunks (bf16, matmul rhs) and c
            hT = [
                state.tile([P, 2], BF16, tag=f"hT{j}", name=f"hT{j}")
                for j in range(NCH)
            ]
            cst = [
                state.tile([P, 2], FP32, tag=f"c{j}", name=f"c{j}")
                for j in range(NCH)
            ]
            # per-chunk gate PSUM tiles [i f o g] x 2 batch cols
            gp = [
                ps.tile([P, 8], FP32, tag=f"gp{j}", bufs=1, name=f"gp{j}")
                for j in range(NCH)
            ]

            def warm(buf):
                # Dummy matmul reading only `buf`: makes the PE observe the
                # buffer's DMA semaphore so real matmuls don't accumulate
                # more sync-waits than the LDWEIGHTS ISA slot budget.
                wp = ps.tile([P, 2], FP32, tag="warmps", bufs=1)
                w = min(buf.shape[1], P)
                nc.tensor.matmul(
                    wp[:w], lhsT=buf[:, 0:w], rhs=buf[:, 0:2], start=True, stop=True
                )

            # load input activations (single DMA: single completion sem)
            nc.sync.dma_start(
                slotA.rearrange("q (k t) -> q k t", k=HB),
                xt_d.rearrange("k q t -> q k t"),
            )
            warm(slotA)

            src, dst = slotA, slotB
            for l in range(n_layers):
                # ---- per-layer weight/bias loads ----
                nc.sync.dma_start(wbuf[:], wxb_d[l])
                warm(wbuf)
                nc.sync.dma_start(whb_sb[:], whb_d[l])
                warm(whb_sb)
                nc.sync.dma_start(bias_sb[:], bb_d[l])
                nc.sync.dma_start(b1_sb[:], b1b_d[l])
                nc.sync.dma_start(b2_sb[:], b2b_d[l])
                if l == 0:
                    # Initialize the scan PSUM tiles' has_written bits: a
                    # start=True dummy covers the tile (clearing+rewriting
                    # its bank's bits), then a start=False dummy re-marks
                    # every byte written so the steady-state flags=0
                    # matmuls always accumulate.
                    for j in range(NCH):
                        nc.tensor.matmul(
                            gp[j], lhsT=wbuf[:, 0:P], rhs=wbuf[:, 0:8],
                            start=True, stop=True, skip_group_check=True,
                        )
                    for j in range(NCH):
                        nc.tensor.matmul(
                            gp[j], lhsT=wbuf[:, 0:P], rhs=wbuf[:, 0:8],
                            start=False, stop=True, skip_group_check=True,
                        )
                tch = tmp.tile([P, 1], FP32, tag="touch")
                nc.vector.tensor_copy(out=tch, in_=bias_sb[:, 0:1])
                nc.vector.tensor_copy(out=tch, in_=b2_sb[:, 0:1])
                tch2 = tmp.tile([P, 1], FP32, tag="touch2")
                nc.scalar.copy(out=tch2, in_=b1_sb[:, 0:1])

                # ---- xg = x @ Wx + b  (gate blocks in chunk-major order) ----
                for (noff, nsz) in NSL:
                    for p in range(MB):
                        pt = pp.tile([P, 512], FP32, tag="ppt")
                        for k in range(KC):
                            nc.tensor.matmul(
                                pt[:, :nsz],
                                lhsT=wbuf[:, (k * MB + p) * P : (k * MB + p + 1) * P],
                                rhs=src[:, k * NTOK + noff : k * NTOK + noff + nsz],
                                start=(k == 0),
                                stop=(k == KC - 1),
                            )
                        # scatter into arena: col = t*32 + p*2 + b
                        out_ap = arena.rearrange("q (t c) -> q t c", c=2 * MB)[
                            :, noff // 2 : (noff + nsz) // 2, 2 * p : 2 * p + 2
                        ]
                        nc.vector.tensor_scalar_add(
                            out=out_ap,
                            in0=pt[:, :nsz].rearrange("q (t c) -> q t c", c=2),
                            scalar1=bias_sb[:, p : p + 1],
                        )

                # ---- LSTM scan over time (chunk-staggered software pipeline) ----
                flip = l % 2 == 1
                for j in range(NCH):
                    nc.vector.memset(cst[j], 0.0)
                    nc.vector.memset(hT[j], 0.0)
                h_view = h_all.rearrange("q (k t) -> q k t", k=HB)
                t0 = (T_steps - 1) if flip else 0
                # preload xg of step 0 into the gate PSUM tiles
                for j in range(NCH):
                    nc.vector.tensor_copy(
                        out=gp[j], in_=arena[:, t0 * 2 * MB + 8 * j : t0 * 2 * MB + 8 * j + 8]
                    )

                for s in range(T_steps):
                    t = (T_steps - 1 - s) if flip else s  # data time index
                    tn = (T_steps - 2 - s) if flip else (s + 1)
                    last = s == T_steps - 1
                    # PE: 16 accumulate-matmuls per chunk, chunks in order.
                    # k outer with per-chunk rotation (chunk j contracts h
                    # chunks in order j, j+1, ..): later-produced h chunks
                    # are consumed later in the stream, shrinking the
                    # step-boundary stall on hT[3].
                    for j in range(NCH):
                        for kk in range(KC):
                            k = (j + kk) % KC
                            for g in range(4):
                                m = j * 4 + g
                                nc.tensor.matmul(
                                    gp[j][:, 2 * g : 2 * g + 2],
                                    lhsT=whb_sb[:, (k * MB + m) * P : (k * MB + m + 1) * P],
                                    rhs=hT[k],
                                    start=False,
                                    stop=(kk == KC - 1),
                                    skip_group_check=True,
                                )
                    # Per-chunk gate tails, software-pipelined.  Both ACT
                    # and DVE queues are strict FIFO, so ops are emitted in
                    # dependency-arrival order: a not-yet-ready op at the
                    # queue head would block later ready ops (head-of-line
                    # blocking).  gp[j] cols: [i i f f o o g g].
                    gt = [
                        tmp.tile([P, 8], FP32, tag=f"gt{j}", name=f"gt{j}")
                        for j in range(NCH)
                    ]
                    th = [
                        tmp.tile([P, 2], FP32, tag=f"th{j}", name=f"th{j}")
                        for j in range(NCH)
                    ]

                    def act_wave(j):
                        nc.scalar.activation(out=gt[j][:, 0:6], in_=gp[j][:, 0:6], func=SIG)
                        nc.scalar.activation(out=gt[j][:, 6:8], in_=gp[j][:, 6:8], func=TANH)

                    def h_out(j):
                        # h chunk j (bf16): the only thing step t+1 needs
                        nc.vector.tensor_mul(out=hT[j], in0=gt[j][:, 4:6], in1=th[j])

                    act_wave(0)
                    act_wave(1)
                    for j in range(NCH):
                        # c_j = sig(f)*c_j + sig(i)*tanh(g)
                        t2 = tmp.tile([P, 2], FP32, tag=f"t2{j}", name=f"t2{j}")
                        nc.vector.tensor_mul(out=t2, in0=gt[j][:, 0:2], in1=gt[j][:, 6:8])
                        nc.vector.tensor_mul(out=cst[j], in0=cst[j], in1=gt[j][:, 2:4])
                        nc.vector.tensor_add(out=cst[j], in0=cst[j], in1=t2)
                        nc.scalar.activation(out=th[j], in_=cst[j], func=TANH)
                        # refill xg for the next step as soon as gp[j]'s
                        # readers (sig/tanh above) are past -- keeps the
                        # preload off the step-boundary critical path
                        if not last:
                            nc.vector.tensor_copy(
                                out=gp[j],
                                in_=arena[:, tn * 2 * MB + 8 * j : tn * 2 * MB + 8 * j + 8],
                            )
                        if j + 2 < NCH:
                            act_wave(j + 2)
                        if j >= 1:
                            h_out(j - 1)
                    h_out(NCH - 1)
                    # h_all writes (off the critical chain), split between
                    # ACT and DVE to balance the two near-saturated engines
                    for j in range(NCH):
                        if j < 2:
                            nc.scalar.copy(
                                out=h_view[:, j, 2 * t : 2 * t + 2], in_=hT[j]
                            )
                        else:
                            nc.vector.tensor_copy(
                                out=h_view[:, j, 2 * t : 2 * t + 2], in_=hT[j]
                            )

                # ---- FFN phase A: z = relu(h @ W1 + b1) ----
                nc.sync.dma_start(wbuf[:], w1b_d[l])
                warm(wbuf)
                for (noff, nsz) in NSL:
                    for p in range(FB):
                        pt = pp.tile([P, 512], FP32, tag="ppt")
                        for k in range(KC):
                            nc.tensor.matmul(
                                pt[:, :nsz],
                                lhsT=wbuf[:, (k * FB + p) * P : (k * FB + p + 1) * P],
                                rhs=h_all[:, k * NTOK + noff : k * NTOK + noff + nsz],
                                start=(k == 0),
                                stop=(k == KC - 1),
                            )
                        nc.scalar.activation(
                            out=arena[:, p * NTOK + noff : p * NTOK + noff + nsz],
                            in_=pt[:, :nsz],
                            func=RELU,
                            bias=b1_sb[:, p : p + 1],
                        )

                # ---- FFN phase B: y = z @ W2 + b2 ----
                nc.sync.dma_start(wbuf[:], w2b_d[l])
                warm(wbuf)
                for (noff, nsz) in NSL:
                    for m in range(HB):
                        pt = pp.tile([P, 512], FP32, tag="ppt")
                        for k in range(FB):
                            nc.tensor.matmul(
                                pt[:, :nsz],
                                lhsT=wbuf[:, (k * HB + m) * P : (k * HB + m + 1) * P],
                                rhs=arena[:, k * NTOK + noff : k * NTOK + noff + nsz],
                                start=(k == 0),
                                stop=(k == FB - 1),
                            )
                        nc.vector.tensor_scalar_add(
                            out=dst[:, m * NTOK + noff : m * NTOK + noff + nsz],
                            in0=pt[:, :nsz],
                            scalar1=b2_sb[:, m : m + 1],
                        )

                src, dst = dst, src

            for r in range(HB):
                nc.sync.dma_start(out_d[r], src[:, r * NTOK : (r + 1) * NTOK])

    nc.compile()
    return nc


# ---------------- host-side data prep ----------------


def _prep_gate_blocks(W: np.ndarray, reorder: bool) -> np.ndarray:
    """[K*P, M*P] -> [P, KC*Mblocks*P] block layout for stationary lhsT use.

    Row r of the result holds, for block (k, p), the weight W[k*P + r, col]
    at free position (k*Mb + p)*P + c.  `reorder` applies the chunk-major
    [i f o g] gate permutation.
    """
    KP, MP = W.shape
    kc, mb = KP // P, MP // P
    v = W.reshape(kc, P, mb, P)
    if reorder:
        v = v[:, :, PERM, :]
    return np.ascontiguousarray(v.transpose(1, 0, 2, 3).reshape(P, kc * mb * P)).astype(
        ml_dtypes.bfloat16
    )


def _prep_bias(b: np.ndarray, reorder: bool) -> np.ndarray:
    """[M*P] -> [P, Mblocks] per-partition bias columns."""
    mb = b.shape[0] // P
    v = b.reshape(mb, P)
    if reorder:
        v = v[PERM]
    return np.ascontiguousarray(v.T).astype(np.float32)


def prep_weights(Wx, Wh, b, W1, b1, W2, b2, n_layers):
    whb = np.stack([_prep_gate_blocks(Wh[l], True) for l in range(n_layers)])
    wxb = np.stack([_prep_gate_blocks(Wx[l], True) for l in range(n_layers)])
    w1b = np.stack([_prep_gate_blocks(W1[l], False) for l in range(n_layers)])
    w2b = np.stack([_prep_gate_blocks(W2[l], False) for l in range(n_layers)])
    bb = np.stack([_prep_bias(b[l], True) for l in range(n_layers)])
    b1b = np.stack([_prep_bias(b1[l], False) for l in range(n_layers)])
    b2b = np.stack([_prep_bias(b2[l], False) for l in range(n_layers)])
    return dict(whb=whb, wxb=wxb, w1b=w1b, w2b=w2b, bb=bb, b1b=b1b, b2b=b2b)


def prep_x_core(x_c: np.ndarray) -> np.ndarray:
    """[BL, T, H] -> [HB, P, BL*T] transposed, tokens (t*2+b)-interleaved."""
    bl, t, h = x_c.shape
    v = x_c.transpose(2, 1, 0).reshape(h, t * bl)  # [H, T*BL] t-major b-minor
    return np.ascontiguousarray(v.reshape(HB, P, t * bl)).astype(ml_dtypes.bfloat16)


def unprep_out_core(o: np.ndarray, t_steps: int) -> np.ndarray:
    """[HB, P, BL*T] -> [BL, T, H]."""
    v = np.asarray(o, dtype=np.float32).reshape(H, t_steps, BL)
    return np.ascontiguousarray(v.transpose(2, 1, 0))


_NC_CACHE = {}


def run_cores(inputs: dict, t_steps=T, n_layers=L, trace=False):
    """Shard inputs, run the SPMD kernel on all 8 cores, return per-core
    outputs plus the raw BassKernelResults (for profiling)."""
    from concourse.bass_utils import run_bass_kernel_spmd

    x = np.asarray(inputs["x"], np.float32)
    wd = prep_weights(
        np.asarray(inputs["Wx"], np.float32),
        np.asarray(inputs["Wh"], np.float32),
        np.asarray(inputs["b"], np.float32),
        np.asarray(inputs["W1"], np.float32),
        np.asarray(inputs["b1"], np.float32),
        np.asarray(inputs["W2"], np.float32),
        np.asarray(inputs["b2"], np.float32),
        n_layers,
    )
    in_maps = []
    for c in range(NCORES):
        m = dict(wd)
        m["xt"] = prep_x_core(x[c * BL : (c + 1) * BL])
        in_maps.append(m)

    key = (t_steps, n_layers)
    if key not in _NC_CACHE:
        _NC_CACHE[key] = _build_nc(t_steps, n_layers)
    nc = _NC_CACHE[key]
    res = run_bass_kernel_spmd(nc, in_maps, core_ids=list(range(NCORES)), trace=trace)
    outs = [unprep_out_core(res.results[c]["out"], t_steps) for c in range(NCORES)]
    return np.concatenate(outs, axis=0), res


def kernel(**inputs) -> np.ndarray:
    out, _ = run_cores(inputs)
    return out.astype(np.float32)


# revision 23
# speedup vs baseline: 1.5378x; 1.0005x over previous
"""Trainium2 Bass kernel: 4-layer alternating-direction LSTM encoder with
per-layer FFN.  Data-parallel over batch across 8 NeuronCores.

v2 design notes (all-bf16 + chunk-staggered scan):
 - Everything "transposed": feature dim on partitions, tokens interleaved
   (t*2 + b) on the free axis, b in {0,1} local batch.
 - All weights/activations bf16 (fp32 PSUM accumulation); rel err ~0.7%.
 - Gate blocks are ordered chunk-major [i_j f_j o_j g_j] for hidden chunk
   j in {0..3}: block position m = j*4 + g maps to natural block
   perm[m] = [j, 4+j, 12+j, 8+j][g].  Each chunk owns one PSUM tile
   [128,8] with cols [i(2) f(2) o(2) g(2)] so sigmoid covers cols 0:6 and
   tanh cols 6:8 in single ACT ops.
 - The recurrent step never issues start=True matmuls: xg_t is DVE-copied
   into the PSUM tile beforehand (has_written bits are set once by init
   dummies), and all 16 matmuls per chunk accumulate onto it.  This both
   removes the xg adds from the critical chain and makes the accumulation
   order-independent.
 - Chunk j's matmuls are emitted as pairs 16j..16j+15 of the step; its
   tail produces h-chunk j (hT[j]) ~2.1us later, just in time for step
   t+1's k=j matmuls (pair 16j) -- a software-pipelined stagger that
   hides most of the serial gate-math chain behind PE work.
 - Sequence flips (odd layers) handled by negated time indexing only.
"""

import os
import sys

sys.path.insert(0, "/opt/trn_rl_repo")

import numpy as np
import ml_dtypes

import concourse.bass as bass
import concourse.bacc as bacc
import concourse.tile as tile
from concourse import mybir
from concourse.bass import ds

FP32 = mybir.dt.float32
BF16 = mybir.dt.bfloat16

L, H, F = 4, 512, 2048
B, T = 16, 512
NCORES = 8
BL = B // NCORES  # local batch per core
P = 128
KC = H // P  # 4 contraction chunks of H
MB = 4 * H // P  # 16 gate blocks
FB = F // P  # 16 filter blocks
HB = H // P  # 4 hidden blocks
NCH = 4  # hidden chunks in the scan
# chunk-major gate order: position m = j*4 + g, g in [i, f, o, g]
PERM = [g4 + j for j in range(4) for g4 in (0, 4, 12, 8)]
# PERM[j*4+0]=j (i), [..+1]=4+j (f), [..+2]=12+j (o), [..+3]=8+j (g)

SIG = mybir.ActivationFunctionType.Sigmoid
TANH = mybir.ActivationFunctionType.Tanh
RELU = mybir.ActivationFunctionType.Relu


def _build_nc(T_steps: int, n_layers: int) -> bass.Bass:
    """Build the per-core Bass program (identical on all cores)."""
    NTOK = BL * T_steps
    NSL = []
    off = 0
    while off < NTOK:
        sz = min(512, NTOK - off)
        NSL.append((off, sz))
        off += sz

    nc = bacc.Bacc()

    xt_d = nc.dram_tensor("xt", [HB, P, NTOK], BF16, kind="ExternalInput")
    whb_d = nc.dram_tensor("whb", [n_layers, P, KC * MB * P], BF16, kind="ExternalInput")
    wxb_d = nc.dram_tensor("wxb", [n_layers, P, KC * MB * P], BF16, kind="ExternalInput")
    w1b_d = nc.dram_tensor("w1b", [n_layers, P, KC * FB * P], BF16, kind="ExternalInput")
    w2b_d = nc.dram_tensor("w2b", [n_layers, P, FB * HB * P], BF16, kind="ExternalInput")
    bb_d = nc.dram_tensor("bb", [n_layers, P, MB], FP32, kind="ExternalInput")
    b1b_d = nc.dram_tensor("b1b", [n_layers, P, FB], FP32, kind="ExternalInput")
    b2b_d = nc.dram_tensor("b2b", [n_layers, P, HB], FP32, kind="ExternalInput")
    out_d = nc.dram_tensor("out", [HB, P, NTOK], BF16, kind="ExternalOutput")

    with tile.TileContext(nc) as tc:
        with (
            tc.tile_pool(name="state", bufs=1) as state,
            tc.tile_pool(name="tmp", bufs=3) as tmp,
            tc.tile_pool(name="psumG", bufs=2, space="PSUM") as pp,
            tc.tile_pool(name="psumS", bufs=1, space="PSUM") as ps,
        ):
            slotA = state.tile([P, HB * NTOK], BF16, tag="slotA")
            slotB = state.tile([P, HB * NTOK], BF16, tag="slotB")
            arena = state.tile([P, T_steps * 2 * MB], BF16, tag="arena")  # xg | z
            h_all = state.tile([P, HB * NTOK], BF16, tag="h_all")
            wbuf = state.tile([P, KC * MB * P], BF16, tag="wbuf")
            whb_sb = state.tile([P, KC * MB * P], BF16, tag="whb_sb")
            bias_sb = state.tile([P, MB], FP32, tag="bias_sb")
            b1_sb = state.tile([P, FB], FP32, tag="b1_sb")
            b2_sb = state.tile([P, HB], FP32, tag="b2_sb")
            # per-chunk recurrent state: h chunks (bf16, matmul rhs) and c
            hT = [
                state.tile([P, 2], BF16, tag=f"hT{j}", name=f"hT{j}")
                for j in range(NCH)
            ]
            cst = [
                state.tile([P, 2], FP32, tag=f"c{j}", name=f"c{j}")
                for j in range(NCH)
            ]
            # per-chunk gate PSUM tiles [i f o g] x 2 batch cols
            gp = [
                ps.tile([P, 8], FP32, tag=f"gp{j}", bufs=1, name=f"gp{j}")
                for j in range(NCH)
            ]

            def warm(buf):
                # Dummy matmul reading only `buf`: makes the PE observe the
                # buffer's DMA semaphore so real matmuls don't accumulate
                # more sync-waits than the LDWEIGHTS ISA slot budget.
                wp = ps.tile([P, 2], FP32, tag="warmps", bufs=1)
                w = min(buf.shape[1], P)
                nc.tensor.matmul(
                    wp[:w], lhsT=buf[:, 0:w], rhs=buf[:, 0:2], start=True, stop=True
                )

            # load input activations (single DMA: single completion sem)
            nc.sync.dma_start(
                slotA.rearrange("q (k t) -> q k t", k=HB),
                xt_d.rearrange("k q t -> q k t"),
            )
            warm(slotA)

            src, dst = slotA, slotB
            for l in range(n_layers):
                # ---- per-layer weight/bias loads ----
                nc.sync.dma_start(wbuf[:], wxb_d[l])
                warm(wbuf)
                nc.sync.dma_start(whb_sb[:], whb_d[l])
                warm(whb_sb)
                nc.sync.dma_start(bias_sb[:], bb_d[l])
                nc.sync.dma_start(b1_sb[:], b1b_d[l])
                nc.sync.dma_start(b2_sb[:], b2b_d[l])
                if l == 0:
                    # Initialize the scan PSUM tiles' has_written bits: a
                    # start=True dummy covers the tile (clearing+rewriting
                    # its bank's bits), then a start=False dummy re-marks
                    # every byte written so the steady-state flags=0
                    # matmuls always accumulate.
                    for j in range(NCH):
                        nc.tensor.matmul(
                            gp[j], lhsT=wbuf[:, 0:P], rhs=wbuf[:, 0:8],
                            start=True, stop=True, skip_group_check=True,
                        )
                    for j in range(NCH):
                        nc.tensor.matmul(
                            gp[j], lhsT=wbuf[:, 0:P], rhs=wbuf[:, 0:8],
                            start=False, stop=True, skip_group_check=True,
                        )
                tch = tmp.tile([P, 1], FP32, tag="touch")
                nc.vector.tensor_copy(out=tch, in_=bias_sb[:, 0:1])
                nc.vector.tensor_copy(out=tch, in_=b2_sb[:, 0:1])
                tch2 = tmp.tile([P, 1], FP32, tag="touch2")
                nc.scalar.copy(out=tch2, in_=b1_sb[:, 0:1])

                # ---- xg = x @ Wx + b  (gate blocks in chunk-major order) ----
                for (noff, nsz) in NSL:
                    for p in range(MB):
                        pt = pp.tile([P, 512], FP32, tag="ppt")
                        for k in range(KC):
                            nc.tensor.matmul(
                                pt[:, :nsz],
                                lhsT=wbuf[:, (k * MB + p) * P : (k * MB + p + 1) * P],
                                rhs=src[:, k * NTOK + noff : k * NTOK + noff + nsz],
                                start=(k == 0),
                                stop=(k == KC - 1),
                            )
                        # scatter into arena: col = t*32 + p*2 + b
                        out_ap = arena.rearrange("q (t c) -> q t c", c=2 * MB)[
                            :, noff // 2 : (noff + nsz) // 2, 2 * p : 2 * p + 2
                        ]
                        nc.vector.tensor_scalar_add(
                            out=out_ap,
                            in0=pt[:, :nsz].rearrange("q (t c) -> q t c", c=2),
                            scalar1=bias_sb[:, p : p + 1],
                        )

                # ---- LSTM scan over time (chunk-staggered software pipeline) ----
                flip = l % 2 == 1
                for j in range(NCH):
                    nc.vector.memset(cst[j], 0.0)
                    nc.vector.memset(hT[j], 0.0)
                h_view = h_all.rearrange("q (k t) -> q k t", k=HB)
                t0 = (T_steps - 1) if flip else 0
                # preload xg of step 0 into the gate PSUM tiles
                for j in range(NCH):
                    nc.vector.tensor_copy(
                        out=gp[j], in_=arena[:, t0 * 2 * MB + 8 * j : t0 * 2 * MB + 8 * j + 8]
                    )

                for s in range(T_steps):
                    t = (T_steps - 1 - s) if flip else s  # data time index
                    tn = (T_steps - 2 - s) if flip else (s + 1)
                    last = s == T_steps - 1
                    # PE: 16 accumulate-matmuls per chunk, chunks in order.
                    # k outer with per-chunk rotation (chunk j contracts h
                    # chunks in order j, j+1, ..): later-produced h chunks
                    # are consumed later in the stream, shrinking the
                    # step-boundary stall on hT[3].
                    for j in range(NCH):
                        for kk in range(KC):
                            k = (j + kk) % KC
                            for g in range(4):
                                m = j * 4 + g
                                nc.tensor.matmul(
                                    gp[j][:, 2 * g : 2 * g + 2],
                                    lhsT=whb_sb[:, (k * MB + m) * P : (k * MB + m + 1) * P],
                                    rhs=hT[k],
                                    start=False,
                                    stop=(kk == KC - 1),
                                    skip_group_check=True,
                                )
                    # Per-chunk gate tails, software-pipelined.  Both ACT
                    # and DVE queues are strict FIFO, so ops are emitted in
                    # dependency-arrival order: a not-yet-ready op at the
                    # queue head would block later ready ops (head-of-line
                    # blocking).  gp[j] cols: [i i f f o o g g].
                    gt = [
                        tmp.tile([P, 8], FP32, tag=f"gt{j}", name=f"gt{j}")
                        for j in range(NCH)
                    ]
                    th = [
                        tmp.tile([P, 2], FP32, tag=f"th{j}", name=f"th{j}")
                        for j in range(NCH)
                    ]

                    def act_wave(j):
                        nc.scalar.activation(out=gt[j][:, 0:6], in_=gp[j][:, 0:6], func=SIG)
                        nc.scalar.activation(out=gt[j][:, 6:8], in_=gp[j][:, 6:8], func=TANH)

                    def h_out(j):
                        # h chunk j (bf16): the only thing step t+1 needs
                        nc.vector.tensor_mul(out=hT[j], in0=gt[j][:, 4:6], in1=th[j])

                    act_wave(0)
                    act_wave(1)
                    for j in range(NCH):
                        # c_j = sig(f)*c_j + sig(i)*tanh(g)
                        t2 = tmp.tile([P, 2], FP32, tag=f"t2{j}", name=f"t2{j}")
                        nc.vector.tensor_mul(out=t2, in0=gt[j][:, 0:2], in1=gt[j][:, 6:8])
                        nc.vector.tensor_mul(out=cst[j], in0=cst[j], in1=gt[j][:, 2:4])
                        nc.vector.tensor_add(out=cst[j], in0=cst[j], in1=t2)
                        nc.scalar.activation(out=th[j], in_=cst[j], func=TANH)
                        # h16 of the previous chunk first: it is on the
                        # h-production critical path, while the xg preload
                        # has a whole step of slack
                        if j >= 1:
                            h_out(j - 1)
                        # refill xg for the next step as soon as gp[j]'s
                        # readers (sig/tanh above) are past -- keeps the
                        # preload off the step-boundary critical path
                        if not last:
                            nc.vector.tensor_copy(
                                out=gp[j],
                                in_=arena[:, tn * 2 * MB + 8 * j : tn * 2 * MB + 8 * j + 8],
                            )
                        if j + 2 < NCH:
                            act_wave(j + 2)
                    h_out(NCH - 1)
                    # h_all writes (off the critical chain), split between
                    # ACT and DVE to balance the two near-saturated engines
                    for j in range(NCH):
                        if j < 2:
                            nc.scalar.copy(
                                out=h_view[:, j, 2 * t : 2 * t + 2], in_=hT[j]
                            )
                        else:
                            nc.vector.tensor_copy(
                                out=h_view[:, j, 2 * t : 2 * t + 2], in_=hT[j]
                            )

                # ---- FFN phase A: z = relu(h @ W1 + b1) ----
                nc.sync.dma_start(wbuf[:], w1b_d[l])
                warm(wbuf)
                for (noff, nsz) in NSL:
                    for p in range(FB):
                        pt = pp.tile([P, 512], FP32, tag="ppt")
                        for k in range(KC):
                            nc.tensor.matmul(
                                pt[:, :nsz],
                                lhsT=wbuf[:, (k * FB + p) * P : (k * FB + p + 1) * P],
                                rhs=h_all[:, k * NTOK + noff : k * NTOK + noff + nsz],
                                start=(k == 0),
                                stop=(k == KC - 1),
                            )
                        nc.scalar.activation(
                            out=arena[:, p * NTOK + noff : p * NTOK + noff + nsz],
                            in_=pt[:, :nsz],
                            func=RELU,
                            bias=b1_sb[:, p : p + 1],
                        )

                # ---- FFN phase B: y = z @ W2 + b2 ----
                nc.sync.dma_start(wbuf[:], w2b_d[l])
                warm(wbuf)
                for (noff, nsz) in NSL:
                    for m in range(HB):
                        pt = pp.tile([P, 512], FP32, tag="ppt")
                        for k in range(FB):
                            nc.tensor.matmul(
                                pt[:, :nsz],
                                lhsT=wbuf[:, (k * HB + m) * P : (k * HB + m + 1) * P],
                                rhs=arena[:, k * NTOK + noff : k * NTOK + noff + nsz],
                                start=(k == 0),
                                stop=(k == FB - 1),
                            )
                        nc.vector.tensor_scalar_add(
                            out=dst[:, m * NTOK + noff : m * NTOK + noff + nsz],
                            in0=pt[:, :nsz],
                            scalar1=b2_sb[:, m : m + 1],
                        )

                src, dst = dst, src

            for r in range(HB):
                nc.sync.dma_start(out_d[r], src[:, r * NTOK : (r + 1) * NTOK])

    nc.compile()
    return nc


# ---------------- host-side data prep ----------------


def _prep_gate_blocks(W: np.ndarray, reorder: bool) -> np.ndarray:
    """[K*P, M*P] -> [P, KC*Mblocks*P] block layout for stationary lhsT use.

    Row r of the result holds, for block (k, p), the weight W[k*P + r, col]
    at free position (k*Mb + p)*P + c.  `reorder` applies the chunk-major
    [i f o g] gate permutation.
    """
    KP, MP = W.shape
    kc, mb = KP // P, MP // P
    v = W.reshape(kc, P, mb, P)
    if reorder:
        v = v[:, :, PERM, :]
    return np.ascontiguousarray(v.transpose(1, 0, 2, 3).reshape(P, kc * mb * P)).astype(
        ml_dtypes.bfloat16
    )


def _prep_bias(b: np.ndarray, reorder: bool) -> np.ndarray:
    """[M*P] -> [P, Mblocks] per-partition bias columns."""
    mb = b.shape[0] // P
    v = b.reshape(mb, P)
    if reorder:
        v = v[PERM]
    return np.ascontiguousarray(v.T).astype(np.float32)


def prep_weights(Wx, Wh, b, W1, b1, W2, b2, n_layers):
    whb = np.stack([_prep_gate_blocks(Wh[l], True) for l in range(n_layers)])
    wxb = np.stack([_prep_gate_blocks(Wx[l], True) for l in range(n_layers)])
    w1b = np.stack([_prep_gate_blocks(W1[l], False) for l in range(n_layers)])
    w2b = np.stack([_prep_gate_blocks(W2[l], False) for l in range(n_layers)])
    bb = np.stack([_prep_bias(b[l], True) for l in range(n_layers)])
    b1b = np.stack([_prep_bias(b1[l], False) for l in range(n_layers)])
    b2b = np.stack([_prep_bias(b2[l], False) for l in range(n_layers)])
    return dict(whb=whb, wxb=wxb, w1b=w1b, w2b=w2b, bb=bb, b1b=b1b, b2b=b2b)


def prep_x_core(x_c: np.ndarray) -> np.ndarray:
    """[BL, T, H] -> [HB, P, BL*T] transposed, tokens (t*2+b)-interleaved."""
    bl, t, h = x_c.shape
    v = x_c.transpose(2, 1, 0).reshape(h, t * bl)  # [H, T*BL] t-major b-minor
    return np.ascontiguousarray(v.reshape(HB, P, t * bl)).astype(ml_dtypes.bfloat16)


def unprep_out_core(o: np.ndarray, t_steps: int) -> np.ndarray:
    """[HB, P, BL*T] -> [BL, T, H]."""
    v = np.asarray(o, dtype=np.float32).reshape(H, t_steps, BL)
    return np.ascontiguousarray(v.transpose(2, 1, 0))


_NC_CACHE = {}


def run_cores(inputs: dict, t_steps=T, n_layers=L, trace=False):
    """Shard inputs, run the SPMD kernel on all 8 cores, return per-core
    outputs plus the raw BassKernelResults (for profiling)."""
    from concourse.bass_utils import run_bass_kernel_spmd

    x = np.asarray(inputs["x"], np.float32)
    wd = prep_weights(
        np.asarray(inputs["Wx"], np.float32),
        np.asarray(inputs["Wh"], np.float32),
        np.asarray(inputs["b"], np.float32),
        np.asarray(inputs["W1"], np.float32),
        np.asarray(inputs["b1"], np.float32),
        np.asarray(inputs["W2"], np.float32),
        np.asarray(inputs["b2"], np.float32),
        n_layers,
    )
    in_maps = []
    for c in range(NCORES):
        m = dict(wd)
        m["xt"] = prep_x_core(x[c * BL : (c + 1) * BL])
        in_maps.append(m)

    key = (t_steps, n_layers)
    if key not in _NC_CACHE:
        _NC_CACHE[key] = _build_nc(t_steps, n_layers)
    nc = _NC_CACHE[key]
    res = run_bass_kernel_spmd(nc, in_maps, core_ids=list(range(NCORES)), trace=trace)
    outs = [unprep_out_core(res.results[c]["out"], t_steps) for c in range(NCORES)]
    return np.concatenate(outs, axis=0), res


def kernel(**inputs) -> np.ndarray:
    out, _ = run_cores(inputs)
    return out.astype(np.float32)
